# revision 16
# baseline (speedup 1.0000x reference)
"""Trainium2 Bass kernel for nn_AppearanceComposability (sparse_attention).

Reference semantics, per (b, c) with 64x64 images, 3x3 unfold (pad 1):
  key_uf  = unfold(key)[b]  : [C*9, 4096]   (channel order (C, kh, kw))
  out     = key_uf.view(C, 4096, 9) * query_uf.view(C, 4096, 9)[..., 4:5]
The raw .view interleave means, with K_flat = per-channel flattened patch
block (kk*4096 + l) and similarly Q_flat:
  out_flat[m] = K_flat[m] * qv[m // 9],   qv[i] = Q_flat[9*i + 4]

Implementation, per NeuronCore (one batch of 8), per 128-channel group
(channels on partitions):
  - load key/query 64x64 images into zero-margined SBUF buffers
  - qv built with 9 stride-9 copies from the query buffer (margins supply
    the unfold zero padding) + small strided memsets for the x-edge wraps
  - per kk chunk: one tensor_tensor multiply of the shifted key image with
    a repeat-9 "stretched" qv (step-0 access pattern), group-aligned with
    margin over/underhang; then a stride-64 memset for x-edge columns
  - 9 stores of [128, 4096] per group to the output chunk

Data parallel over batch: 8 cores, core b handles batch b. No collectives.
"""
import os
import sys

import numpy as np


def _ensure_path():
    try:
        import concourse  # noqa: F401
    except ImportError:
        for p in ("/opt/trn_rl_repo", "/root/.axon_site/_ro/trn_rl_repo"):
            if os.path.isdir(p):
                sys.path.insert(0, p)
                return


_ensure_path()

import concourse.bacc as bacc  # noqa: E402
import concourse.tile as tile  # noqa: E402
from concourse import mybir  # noqa: E402
from concourse.bass_utils import run_bass_kernel_spmd  # noqa: E402
from concourse.tile import add_dep_helper  # noqa: E402


def _install_ntff_hook_shim():
    """Provide antenv.axon_hooks when the image's antenv lacks it.

    concourse.bass_utils imports it unconditionally on the trace path; the
    boot script degrades silently when it is missing. This shim recreates
    the documented hook using the same ctypes loader the boot script uses.
    """
    try:
        import antenv.axon_hooks  # noqa: F401
        return
    except ImportError:
        pass
    try:
        import types

        import antenv
        holder = {"hook": None, "tried": False}

        def set_axon_ntff_profile_hook(h):
            holder["hook"] = h
            holder["tried"] = True

        def get_axon_ntff_profile_hook():
            if not holder["tried"]:
                holder["tried"] = True
                try:
                    from trn_agent_boot.trn_boot import _ntff_profile_via_ctypes
                    so = "/opt/axon/libaxon_pjrt.so"
                    if os.path.exists(so):
                        holder["hook"] = _ntff_profile_via_ctypes(so)
                except Exception:
                    holder["hook"] = None
            return holder["hook"]

        mod = types.ModuleType("antenv.axon_hooks")
        mod.set_axon_ntff_profile_hook = set_axon_ntff_profile_hook
        mod.get_axon_ntff_profile_hook = get_axon_ntff_profile_hook
        sys.modules["antenv.axon_hooks"] = mod
        antenv.axon_hooks = mod
    except Exception:
        pass


_install_ntff_hook_shim()

F32 = mybir.dt.float32

B = 8          # batch == number of cores
C = 256        # channels
H = W = 64
L = H * W      # 4096 pixels
K2 = 9         # 3x3 patch
M = L * K2     # 36864 per-channel output length
MARG = 80      # input image margin (>= 73 needed)
OM = 8         # output tile margin (>= 8 needed)
OFFS = [(kh - 1) * W + (kw - 1) for kh in range(3) for kw in range(3)]

# All multiplies run on DVE. GpSimd tensor_tensor shares an SBUF port pair
# with DVE's second read port; concurrent DVE-TT + GpSimd-TT measured ~2.5x
# slower on both engines (exclusive port lock), so offloading to GpSimd is
# a net loss. DVE alone (~81 us) stays below the DMA floor (~110 us).
POOL_KKS = ()


def _ceil_div(a, b):
    return -(-a // b)


def _plan_qv_ops():
    """Per kk: (i_lo, i_hi, src_start, memsets) for qv[i] = Q_flat[9i+4]."""
    ops = []
    for kk in range(K2):
        s = L * kk
        i_lo = max(0, _ceil_div(s - 4, 9))
        i_hi = min(L, _ceil_div(s + L - 4, 9))
        src_start = 9 * i_lo + 4 - s + OFFS[kk]
        memsets = []
        kw = kk % 3
        if kw != 1:
            target = 0 if kw == 0 else 63
            i0 = (57 * (target - 4 + s)) % 64  # 57 = 9^-1 mod 64
            first = i_lo + ((i0 - i_lo) % 64)
            if first < i_hi:
                cnt = (i_hi - 1 - first) // 64 + 1
                memsets.append((first, cnt, 64))
        ops.append((i_lo, i_hi, src_start, memsets))
    return ops


def _plan_tt_ops():
    """Per kk: (g_lo, g_hi, ngroups, q0); TT covers l in [g_lo, g_hi)."""
    ops = []
    for kk in range(K2):
        s = L * kk
        g_lo = -(s % 9)
        g_hi = L + ((-(s + L)) % 9)
        ops.append((g_lo, g_hi, (g_hi - g_lo) // 9, (s + g_lo) // 9))
    return ops


QV_OPS = _plan_qv_ops()
TT_OPS = _plan_tt_ops()


def build_graph():
    nc = bacc.Bacc(None, target_bir_lowering=False)
    key_ext = nc.declare_dram_parameter("key_map", [C, L], F32, isOutput=False)
    query_ext = nc.declare_dram_parameter("query_map", [C, L], F32, isOutput=False)
    out_ext = nc.declare_dram_parameter("out", [C, M], F32, isOutput=True)

    ngroups = C // 128
    with tile.TileContext(nc) as tc:
        with (
            tc.tile_pool(name="pads", bufs=1) as pads,
            tc.tile_pool(name="qvp", bufs=1) as qvp,
            tc.tile_pool(name="outs", bufs=6) as outs,
        ):
            key_pads, q_pads, qvs = [], [], []
            # Phase 0: prefetch every input load, zero all pad margins, and
            # build both groups' qv on ACT — all before any store is queued,
            # so the group transition never starves the DMA rings. Group 1's
            # loads are chained behind group 0's: the SDMA engines interleave
            # concurrent transfers on a ring at packet granularity, so an
            # unchained q1 would delay q0 (and the whole pipeline fill) ~6us.
            prev_load = {}
            for g in range(ngroups):
                rows = slice(g * 128, (g + 1) * 128)
                q_pad = pads.tile([128, MARG + L + MARG], F32,
                                  name=f"q_pad{g}", tag=f"q_pad{g}")
                nc.vector.memset(q_pad[:, 0:MARG], 0.0)
                nc.vector.memset(q_pad[:, MARG + L:MARG + L + MARG], 0.0)
                qdma = nc.sync.dma_start(q_pad[:, MARG:MARG + L],
                                         query_ext[rows, :])
                if "last" in prev_load:
                    add_dep_helper(qdma.ins, prev_load["last"].ins, sync=True,
                                   reason="serialize loads: SDMA engines are "
                                          "shared across rings")
                prev_load["last"] = qdma
                key_pad = pads.tile([128, MARG + L + MARG], F32,
                                    name=f"key_pad{g}", tag=f"key_pad{g}")
                nc.vector.memset(key_pad[:, 0:MARG], 0.0)
                nc.vector.memset(key_pad[:, MARG + L:MARG + L + MARG], 0.0)
                kdma = nc.scalar.dma_start(key_pad[:, MARG:MARG + L],
                                           key_ext[rows, :])
                add_dep_helper(kdma.ins, prev_load["last"].ins, sync=True,
                               reason="serialize loads: SDMA engines are "
                                      "shared across rings")
                prev_load["last"] = kdma
                key_pads.append(key_pad)
                q_pads.append(q_pad)

            for g in range(ngroups):
                qv = qvp.tile([128, L], F32, name=f"qv{g}", tag=f"qv{g}")
                for kk in range(K2):
                    i_lo, i_hi, src_start, _ = QV_OPS[kk]
                    n = i_hi - i_lo
                    src0 = MARG + src_start
                    nc.scalar.copy(qv[:, i_lo:i_hi],
                                   q_pads[g][:, src0:src0 + 9 * n:9])
                qvs.append(qv)

            # Phase 1: per group — the nine chunk multiplies. The qv edge
            # memset for chunk kk is emitted just before TT kk (TT kk's qv
            # read range only ever overlaps chunk kk's own memset), so DVE
            # never stalls waiting for late ACT copies.
            prev_colmset = None
            for g in range(ngroups):
                rows = slice(g * 128, (g + 1) * 128)
                key_pad, qv = key_pads[g], qvs[g]

                for kk in range(K2):
                    for (first, cnt, stride) in QV_OPS[kk][3]:
                        nc.vector.memset(
                            qv[:, first:first + (cnt - 1) * stride + 1:stride],
                            0.0)
                    g_lo, g_hi, ng, q0 = TT_OPS[kk]
                    ot = outs.tile([128, OM + L + OM], F32,
                                   name=f"ot{g}_{kk}", tag="ot")
                    dst = ot[:, OM + g_lo:OM + g_hi].rearrange(
                        "p (n k) -> p n k", k=9)
                    src_k = key_pad[:, MARG + g_lo + OFFS[kk]:
                                    MARG + g_hi + OFFS[kk]].rearrange(
                        "p (n k) -> p n k", k=9)
                    src_q = qv[:, q0:q0 + ng].unsqueeze(2).broadcast_to(
                        [128, ng, 9])
                    eng = nc.gpsimd if kk in POOL_KKS else nc.vector
                    tt = eng.tensor_mul(dst, src_k, src_q)
                    if prev_colmset is not None:
                        # Pin DVE order TT_k -> colmset_k -> TT_{k+1}: the
                        # scheduler otherwise runs the next TT first, holding
                        # the finished tile's store back a full TT (~4.4us).
                        add_dep_helper(tt.ins, prev_colmset.ins, sync=False,
                                       reason="colmset before next TT")
                        prev_colmset = None

                    kw = kk % 3
                    if kw == 0:
                        prev_colmset = nc.vector.memset(
                            ot[:, OM:OM + L:64], 0.0)
                    elif kw == 2:
                        prev_colmset = nc.vector.memset(
                            ot[:, OM + 63:OM + L:64], 0.0)

                    deng = nc.sync if kk % 2 == 0 else nc.scalar
                    if os.environ.get("K_STORE_ENG") == "sync":
                        deng = nc.sync
                    deng.dma_start(out_ext[rows, kk * L:(kk + 1) * L],
                                   ot[:, OM:OM + L])
    nc.compile()
    return nc


_GRAPH_CACHE = {}


def _get_graph():
    if "nc" not in _GRAPH_CACHE:
        _GRAPH_CACHE["nc"] = build_graph()
    return _GRAPH_CACHE["nc"]


def kernel(key_map: np.ndarray, query_map: np.ndarray,
           _trace: bool = False, _tmpdir: str | None = None):
    key_map = np.ascontiguousarray(key_map, dtype=np.float32)
    query_map = np.ascontiguousarray(query_map, dtype=np.float32)
    assert key_map.shape == (B, C, H, W), key_map.shape

    nc = _get_graph()
    in_maps = [
        {"key_map": key_map[b].reshape(C, L),
         "query_map": query_map[b].reshape(C, L)}
        for b in range(B)
    ]
    res = run_bass_kernel_spmd(
        nc, in_maps, core_ids=list(range(B)),
        trace=_trace, tmpdir=_tmpdir,
    )
    out = np.stack([res.results[b]["out"] for b in range(B)])
    _GRAPH_CACHE["last_exec_time_ns"] = res.exec_time_ns
    _GRAPH_CACHE["last_results"] = res
    return out.reshape(B, C, L, K2)


# revision 18
# speedup vs baseline: 1.1694x; 1.1694x over previous
"""Trainium2 Bass kernel for nn_AppearanceComposability (sparse_attention).

Reference semantics, per (b, c) with 64x64 images, 3x3 unfold (pad 1):
  key_uf  = unfold(key)[b]  : [C*9, 4096]   (channel order (C, kh, kw))
  out     = key_uf.view(C, 4096, 9) * query_uf.view(C, 4096, 9)[..., 4:5]
The raw .view interleave means, with K_flat = per-channel flattened patch
block (kk*4096 + l) and similarly Q_flat:
  out_flat[m] = K_flat[m] * qv[m // 9],   qv[i] = Q_flat[9*i + 4]

Implementation, per NeuronCore (one batch of 8), per 128-channel group
(channels on partitions):
  - load key/query 64x64 images into zero-margined SBUF buffers
  - qv built with 9 stride-9 copies from the query buffer (margins supply
    the unfold zero padding) + small strided memsets for the x-edge wraps
  - per kk chunk: one tensor_tensor multiply of the shifted key image with
    a repeat-9 "stretched" qv (step-0 access pattern), group-aligned with
    margin over/underhang; then a stride-64 memset for x-edge columns
  - 9 stores of [128, 4096] per group to the output chunk

Data parallel over batch: 8 cores, core b handles batch b. No collectives.
"""
import os
import sys

import numpy as np


def _ensure_path():
    try:
        import concourse  # noqa: F401
    except ImportError:
        for p in ("/opt/trn_rl_repo", "/root/.axon_site/_ro/trn_rl_repo"):
            if os.path.isdir(p):
                sys.path.insert(0, p)
                return


_ensure_path()

import concourse.bacc as bacc  # noqa: E402
import concourse.tile as tile  # noqa: E402
from concourse import mybir  # noqa: E402
from concourse.bass_utils import run_bass_kernel_spmd  # noqa: E402
from concourse.tile import add_dep_helper  # noqa: E402


def _install_ntff_hook_shim():
    """Provide antenv.axon_hooks when the image's antenv lacks it.

    concourse.bass_utils imports it unconditionally on the trace path; the
    boot script degrades silently when it is missing. This shim recreates
    the documented hook using the same ctypes loader the boot script uses.
    """
    try:
        import antenv.axon_hooks  # noqa: F401
        return
    except ImportError:
        pass
    try:
        import types

        import antenv
        holder = {"hook": None, "tried": False}

        def set_axon_ntff_profile_hook(h):
            holder["hook"] = h
            holder["tried"] = True

        def get_axon_ntff_profile_hook():
            if not holder["tried"]:
                holder["tried"] = True
                try:
                    from trn_agent_boot.trn_boot import _ntff_profile_via_ctypes
                    so = "/opt/axon/libaxon_pjrt.so"
                    if os.path.exists(so):
                        holder["hook"] = _ntff_profile_via_ctypes(so)
                except Exception:
                    holder["hook"] = None
            return holder["hook"]

        mod = types.ModuleType("antenv.axon_hooks")
        mod.set_axon_ntff_profile_hook = set_axon_ntff_profile_hook
        mod.get_axon_ntff_profile_hook = get_axon_ntff_profile_hook
        sys.modules["antenv.axon_hooks"] = mod
        antenv.axon_hooks = mod
    except Exception:
        pass


_install_ntff_hook_shim()

F32 = mybir.dt.float32

B = 8          # batch == number of cores
C = 256        # channels
H = W = 64
L = H * W      # 4096 pixels
K2 = 9         # 3x3 patch
M = L * K2     # 36864 per-channel output length
MARG = 80      # input image margin (>= 73 needed)
OM = 8         # output tile margin (>= 8 needed)
OFFS = [(kh - 1) * W + (kw - 1) for kh in range(3) for kw in range(3)]

# All multiplies run on DVE. GpSimd tensor_tensor shares an SBUF port pair
# with DVE's second read port; concurrent DVE-TT + GpSimd-TT measured ~2.5x
# slower on both engines (exclusive port lock), so offloading to GpSimd is
# a net loss. DVE alone (~81 us) stays below the DMA floor (~110 us).
POOL_KKS = ()


def _ceil_div(a, b):
    return -(-a // b)


def _plan_qv_ops():
    """Per kk: (i_lo, i_hi, src_start, memsets) for qv[i] = Q_flat[9i+4]."""
    ops = []
    for kk in range(K2):
        s = L * kk
        i_lo = max(0, _ceil_div(s - 4, 9))
        i_hi = min(L, _ceil_div(s + L - 4, 9))
        src_start = 9 * i_lo + 4 - s + OFFS[kk]
        memsets = []
        kw = kk % 3
        if kw != 1:
            target = 0 if kw == 0 else 63
            i0 = (57 * (target - 4 + s)) % 64  # 57 = 9^-1 mod 64
            first = i_lo + ((i0 - i_lo) % 64)
            if first < i_hi:
                cnt = (i_hi - 1 - first) // 64 + 1
                memsets.append((first, cnt, 64))
        ops.append((i_lo, i_hi, src_start, memsets))
    return ops


def _plan_tt_ops():
    """Per kk: (g_lo, g_hi, ngroups, q0); TT covers l in [g_lo, g_hi)."""
    ops = []
    for kk in range(K2):
        s = L * kk
        g_lo = -(s % 9)
        g_hi = L + ((-(s + L)) % 9)
        ops.append((g_lo, g_hi, (g_hi - g_lo) // 9, (s + g_lo) // 9))
    return ops


QV_OPS = _plan_qv_ops()
TT_OPS = _plan_tt_ops()


def build_graph():
    nc = bacc.Bacc(None, target_bir_lowering=False)
    key_ext = nc.declare_dram_parameter("key_map", [C, L], F32, isOutput=False)
    query_ext = nc.declare_dram_parameter("query_map", [C, L], F32, isOutput=False)
    out_ext = nc.declare_dram_parameter("out", [C, M], F32, isOutput=True)

    ngroups = C // 128
    with tile.TileContext(nc) as tc:
        with (
            tc.tile_pool(name="pads", bufs=1) as pads,
            tc.tile_pool(name="qvp", bufs=1) as qvp,
            tc.tile_pool(name="outs", bufs=6) as outs,
        ):
            key_pads, q_pads, qvs = [], [], []
            # Phase 0: prefetch every input load, zero all pad margins, and
            # build both groups' qv on ACT — all before any store is queued,
            # so the group transition never starves the DMA rings. Group 1's
            # loads are chained behind group 0's: the SDMA engines interleave
            # concurrent transfers on a ring at packet granularity, so an
            # unchained q1 would delay q0 (and the whole pipeline fill) ~6us.
            prev_load = {}
            for g in range(ngroups):
                rows = slice(g * 128, (g + 1) * 128)
                q_pad = pads.tile([128, MARG + L + MARG], F32,
                                  name=f"q_pad{g}", tag=f"q_pad{g}")
                nc.vector.memset(q_pad[:, 0:MARG], 0.0)
                nc.vector.memset(q_pad[:, MARG + L:MARG + L + MARG], 0.0)
                qdma = nc.sync.dma_start(q_pad[:, MARG:MARG + L],
                                         query_ext[rows, :])
                if "q" in prev_load:
                    # Chain group-1 behind group-0 per ring: concurrent DMAs
                    # split SDMA bandwidth at packet granularity, which would
                    # delay q0 (the pipeline-fill gate). Keep exactly two
                    # transfers in flight (q || k): one alone only reaches
                    # ~240 GB/s; a pair reaches ~420 GB/s aggregate.
                    add_dep_helper(qdma.ins, prev_load["q"].ins, sync=True,
                                   reason="serialize group loads on ring")
                prev_load["q"] = qdma
                key_pad = pads.tile([128, MARG + L + MARG], F32,
                                    name=f"key_pad{g}", tag=f"key_pad{g}")
                nc.vector.memset(key_pad[:, 0:MARG], 0.0)
                nc.vector.memset(key_pad[:, MARG + L:MARG + L + MARG], 0.0)
                kdma = nc.scalar.dma_start(key_pad[:, MARG:MARG + L],
                                           key_ext[rows, :])
                if "k" in prev_load:
                    add_dep_helper(kdma.ins, prev_load["k"].ins, sync=True,
                                   reason="serialize group loads on ring")
                prev_load["k"] = kdma
                key_pads.append(key_pad)
                q_pads.append(q_pad)

            for g in range(ngroups):
                qv = qvp.tile([128, L], F32, name=f"qv{g}", tag=f"qv{g}")
                for kk in range(K2):
                    i_lo, i_hi, src_start, _ = QV_OPS[kk]
                    n = i_hi - i_lo
                    src0 = MARG + src_start
                    nc.scalar.copy(qv[:, i_lo:i_hi],
                                   q_pads[g][:, src0:src0 + 9 * n:9])
                qvs.append(qv)

            # Phase 1: per group — the nine chunk multiplies. The qv edge
            # memset for chunk kk is emitted just before TT kk (TT kk's qv
            # read range only ever overlaps chunk kk's own memset), so DVE
            # never stalls waiting for late ACT copies.
            prev_colmset = None
            for g in range(ngroups):
                rows = slice(g * 128, (g + 1) * 128)
                key_pad, qv = key_pads[g], qvs[g]

                for kk in range(K2):
                    for (first, cnt, stride) in QV_OPS[kk][3]:
                        nc.vector.memset(
                            qv[:, first:first + (cnt - 1) * stride + 1:stride],
                            0.0)
                    g_lo, g_hi, ng, q0 = TT_OPS[kk]
                    ot = outs.tile([128, OM + L + OM], F32,
                                   name=f"ot{g}_{kk}", tag="ot")
                    dst = ot[:, OM + g_lo:OM + g_hi].rearrange(
                        "p (n k) -> p n k", k=9)
                    src_k = key_pad[:, MARG + g_lo + OFFS[kk]:
                                    MARG + g_hi + OFFS[kk]].rearrange(
                        "p (n k) -> p n k", k=9)
                    src_q = qv[:, q0:q0 + ng].unsqueeze(2).broadcast_to(
                        [128, ng, 9])
                    eng = nc.gpsimd if kk in POOL_KKS else nc.vector
                    tt = eng.tensor_mul(dst, src_k, src_q)
                    if prev_colmset is not None:
                        # Pin DVE order TT_k -> colmset_k -> TT_{k+1}: the
                        # scheduler otherwise runs the next TT first, holding
                        # the finished tile's store back a full TT (~4.4us).
                        add_dep_helper(tt.ins, prev_colmset.ins, sync=False,
                                       reason="colmset before next TT")
                        prev_colmset = None

                    kw = kk % 3
                    if kw == 0:
                        prev_colmset = nc.vector.memset(
                            ot[:, OM:OM + L:64], 0.0)
                    elif kw == 2:
                        prev_colmset = nc.vector.memset(
                            ot[:, OM + 63:OM + L:64], 0.0)

                    deng = nc.sync if kk % 2 == 0 else nc.scalar
                    if os.environ.get("K_STORE_ENG") == "sync":
                        deng = nc.sync
                    deng.dma_start(out_ext[rows, kk * L:(kk + 1) * L],
                                   ot[:, OM:OM + L])
    nc.compile()
    return nc


_GRAPH_CACHE = {}


def _get_graph():
    if "nc" not in _GRAPH_CACHE:
        _GRAPH_CACHE["nc"] = build_graph()
    return _GRAPH_CACHE["nc"]


def kernel(key_map: np.ndarray, query_map: np.ndarray,
           _trace: bool = False, _tmpdir: str | None = None):
    key_map = np.ascontiguousarray(key_map, dtype=np.float32)
    query_map = np.ascontiguousarray(query_map, dtype=np.float32)
    assert key_map.shape == (B, C, H, W), key_map.shape

    nc = _get_graph()
    in_maps = [
        {"key_map": key_map[b].reshape(C, L),
         "query_map": query_map[b].reshape(C, L)}
        for b in range(B)
    ]
    res = run_bass_kernel_spmd(
        nc, in_maps, core_ids=list(range(B)),
        trace=_trace, tmpdir=_tmpdir,
    )
    out = np.stack([res.results[b]["out"] for b in range(B)])
    _GRAPH_CACHE["last_exec_time_ns"] = res.exec_time_ns
    _GRAPH_CACHE["last_results"] = res
    return out.reshape(B, C, L, K2)


# revision 19
# speedup vs baseline: 1.2105x; 1.0352x over previous
"""Trainium2 Bass kernel for nn_AppearanceComposability (sparse_attention).

Reference semantics, per (b, c) with 64x64 images, 3x3 unfold (pad 1):
  key_uf  = unfold(key)[b]  : [C*9, 4096]   (channel order (C, kh, kw))
  out     = key_uf.view(C, 4096, 9) * query_uf.view(C, 4096, 9)[..., 4:5]
The raw .view interleave means, with K_flat = per-channel flattened patch
block (kk*4096 + l) and similarly Q_flat:
  out_flat[m] = K_flat[m] * qv[m // 9],   qv[i] = Q_flat[9*i + 4]

Implementation, per NeuronCore (one batch of 8), per 128-channel group
(channels on partitions):
  - load key/query 64x64 images into zero-margined SBUF buffers
  - qv built with 9 stride-9 copies from the query buffer (margins supply
    the unfold zero padding) + small strided memsets for the x-edge wraps
  - per kk chunk: one tensor_tensor multiply of the shifted key image with
    a repeat-9 "stretched" qv (step-0 access pattern), group-aligned with
    margin over/underhang; then a stride-64 memset for x-edge columns
  - 9 stores of [128, 4096] per group to the output chunk

Data parallel over batch: 8 cores, core b handles batch b. No collectives.
"""
import os
import sys

import numpy as np


def _ensure_path():
    try:
        import concourse  # noqa: F401
    except ImportError:
        for p in ("/opt/trn_rl_repo", "/root/.axon_site/_ro/trn_rl_repo"):
            if os.path.isdir(p):
                sys.path.insert(0, p)
                return


_ensure_path()

import concourse.bacc as bacc  # noqa: E402
import concourse.tile as tile  # noqa: E402
from concourse import mybir  # noqa: E402
from concourse.bass_utils import run_bass_kernel_spmd  # noqa: E402
from concourse.tile import add_dep_helper  # noqa: E402


def _install_ntff_hook_shim():
    """Provide antenv.axon_hooks when the image's antenv lacks it.

    concourse.bass_utils imports it unconditionally on the trace path; the
    boot script degrades silently when it is missing. This shim recreates
    the documented hook using the same ctypes loader the boot script uses.
    """
    try:
        import antenv.axon_hooks  # noqa: F401
        return
    except ImportError:
        pass
    try:
        import types

        import antenv
        holder = {"hook": None, "tried": False}

        def set_axon_ntff_profile_hook(h):
            holder["hook"] = h
            holder["tried"] = True

        def get_axon_ntff_profile_hook():
            if not holder["tried"]:
                holder["tried"] = True
                try:
                    from trn_agent_boot.trn_boot import _ntff_profile_via_ctypes
                    so = "/opt/axon/libaxon_pjrt.so"
                    if os.path.exists(so):
                        holder["hook"] = _ntff_profile_via_ctypes(so)
                except Exception:
                    holder["hook"] = None
            return holder["hook"]

        mod = types.ModuleType("antenv.axon_hooks")
        mod.set_axon_ntff_profile_hook = set_axon_ntff_profile_hook
        mod.get_axon_ntff_profile_hook = get_axon_ntff_profile_hook
        sys.modules["antenv.axon_hooks"] = mod
        antenv.axon_hooks = mod
    except Exception:
        pass


_install_ntff_hook_shim()

F32 = mybir.dt.float32

B = 8          # batch == number of cores
C = 256        # channels
H = W = 64
L = H * W      # 4096 pixels
K2 = 9         # 3x3 patch
M = L * K2     # 36864 per-channel output length
MARG = 80      # input image margin (>= 73 needed)
OM = 8         # output tile margin (>= 8 needed)
OFFS = [(kh - 1) * W + (kw - 1) for kh in range(3) for kw in range(3)]

# All multiplies run on DVE. GpSimd tensor_tensor shares an SBUF port pair
# with DVE's second read port; concurrent DVE-TT + GpSimd-TT measured ~2.5x
# slower on both engines (exclusive port lock), so offloading to GpSimd is
# a net loss. DVE alone (~81 us) stays below the DMA floor (~110 us).
POOL_KKS = ()


def _ceil_div(a, b):
    return -(-a // b)


def _plan_qv_ops():
    """Per kk: (i_lo, i_hi, src_start, memsets) for qv[i] = Q_flat[9i+4]."""
    ops = []
    for kk in range(K2):
        s = L * kk
        i_lo = max(0, _ceil_div(s - 4, 9))
        i_hi = min(L, _ceil_div(s + L - 4, 9))
        src_start = 9 * i_lo + 4 - s + OFFS[kk]
        memsets = []
        kw = kk % 3
        if kw != 1:
            target = 0 if kw == 0 else 63
            i0 = (57 * (target - 4 + s)) % 64  # 57 = 9^-1 mod 64
            first = i_lo + ((i0 - i_lo) % 64)
            if first < i_hi:
                cnt = (i_hi - 1 - first) // 64 + 1
                memsets.append((first, cnt, 64))
        ops.append((i_lo, i_hi, src_start, memsets))
    return ops


def _plan_tt_ops():
    """Per kk: (g_lo, g_hi, ngroups, q0); TT covers l in [g_lo, g_hi)."""
    ops = []
    for kk in range(K2):
        s = L * kk
        g_lo = -(s % 9)
        g_hi = L + ((-(s + L)) % 9)
        ops.append((g_lo, g_hi, (g_hi - g_lo) // 9, (s + g_lo) // 9))
    return ops


QV_OPS = _plan_qv_ops()
TT_OPS = _plan_tt_ops()


def build_graph():
    nc = bacc.Bacc(None, target_bir_lowering=False)
    key_ext = nc.declare_dram_parameter("key_map", [C, L], F32, isOutput=False)
    query_ext = nc.declare_dram_parameter("query_map", [C, L], F32, isOutput=False)
    out_ext = nc.declare_dram_parameter("out", [C, M], F32, isOutput=True)

    ngroups = C // 128
    with tile.TileContext(nc) as tc:
        with (
            tc.tile_pool(name="pads", bufs=1) as pads,
            tc.tile_pool(name="qvp", bufs=1) as qvp,
            tc.tile_pool(name="outs", bufs=6) as outs,
        ):
            key_pads, q_pads, qvs = [], [], []
            # Phase 0: prefetch inputs and build both groups' qv on ACT
            # before any store is queued. Ring discipline learned from
            # profiles: (a) concurrent DMAs split the 16 SDMA engines at
            # packet granularity — exactly two in flight (one per HWDGE
            # ring) gives ~420 GB/s aggregate, one alone only ~240; (b) the
            # pipeline-fill gate is q0 (qv copies) then k0 (first TT), so k0
            # is split into two chained halves — q0 then finishes ~3us
            # before the k0 tail instead of tied with it; (c) the second
            # group's load *issues* are placed after the first group's ACT
            # copies so the ACT sequencer is never blocked on a load
            # semaphore ahead of the copies.
            for g in range(ngroups):
                q_pad = pads.tile([128, MARG + L + MARG], F32,
                                  name=f"q_pad{g}", tag=f"q_pad{g}")
                nc.vector.memset(q_pad[:, 0:MARG], 0.0)
                nc.vector.memset(q_pad[:, MARG + L:MARG + L + MARG], 0.0)
                key_pad = pads.tile([128, MARG + L + MARG], F32,
                                    name=f"key_pad{g}", tag=f"key_pad{g}")
                nc.vector.memset(key_pad[:, 0:MARG], 0.0)
                nc.vector.memset(key_pad[:, MARG + L:MARG + L + MARG], 0.0)
                key_pads.append(key_pad)
                q_pads.append(q_pad)

            halfL = L // 2
            q0dma = nc.sync.dma_start(q_pads[0][:, MARG:MARG + L],
                                      query_ext[0:128, :])
            k0a = nc.scalar.dma_start(key_pads[0][:, MARG:MARG + halfL],
                                      key_ext[0:128, 0:halfL])
            k0b = nc.scalar.dma_start(key_pads[0][:, MARG + halfL:MARG + L],
                                      key_ext[0:128, halfL:L])
            add_dep_helper(k0b.ins, k0a.ins, sync=True,
                           reason="chain k0 halves so q0 lands first")

            def emit_qv(g):
                qv = qvp.tile([128, L], F32, name=f"qv{g}", tag=f"qv{g}")
                for kk in range(K2):
                    i_lo, i_hi, src_start, _ = QV_OPS[kk]
                    n = i_hi - i_lo
                    src0 = MARG + src_start
                    nc.scalar.copy(qv[:, i_lo:i_hi],
                                   q_pads[g][:, src0:src0 + 9 * n:9])
                qvs.append(qv)

            emit_qv(0)

            q1dma = nc.sync.dma_start(q_pads[1][:, MARG:MARG + L],
                                      query_ext[128:256, :])
            add_dep_helper(q1dma.ins, q0dma.ins, sync=True,
                           reason="serialize group loads on ring")
            k1dma = nc.scalar.dma_start(key_pads[1][:, MARG:MARG + L],
                                        key_ext[128:256, :])
            add_dep_helper(k1dma.ins, k0b.ins, sync=True,
                           reason="serialize group loads on ring")

            emit_qv(1)

            # Phase 1: per group — the nine chunk multiplies. The qv edge
            # memset for chunk kk is emitted just before TT kk (TT kk's qv
            # read range only ever overlaps chunk kk's own memset), so DVE
            # never stalls waiting for late ACT copies.
            prev_colmset = None
            for g in range(ngroups):
                rows = slice(g * 128, (g + 1) * 128)
                key_pad, qv = key_pads[g], qvs[g]

                for kk in range(K2):
                    for (first, cnt, stride) in QV_OPS[kk][3]:
                        nc.vector.memset(
                            qv[:, first:first + (cnt - 1) * stride + 1:stride],
                            0.0)
                    g_lo, g_hi, ng, q0 = TT_OPS[kk]
                    ot = outs.tile([128, OM + L + OM], F32,
                                   name=f"ot{g}_{kk}", tag="ot")
                    dst = ot[:, OM + g_lo:OM + g_hi].rearrange(
                        "p (n k) -> p n k", k=9)
                    src_k = key_pad[:, MARG + g_lo + OFFS[kk]:
                                    MARG + g_hi + OFFS[kk]].rearrange(
                        "p (n k) -> p n k", k=9)
                    src_q = qv[:, q0:q0 + ng].unsqueeze(2).broadcast_to(
                        [128, ng, 9])
                    eng = nc.gpsimd if kk in POOL_KKS else nc.vector
                    tt = eng.tensor_mul(dst, src_k, src_q)
                    if prev_colmset is not None:
                        # Pin DVE order TT_k -> colmset_k -> TT_{k+1}: the
                        # scheduler otherwise runs the next TT first, holding
                        # the finished tile's store back a full TT (~4.4us).
                        add_dep_helper(tt.ins, prev_colmset.ins, sync=False,
                                       reason="colmset before next TT")
                        prev_colmset = None

                    kw = kk % 3
                    if kw == 0:
                        prev_colmset = nc.vector.memset(
                            ot[:, OM:OM + L:64], 0.0)
                    elif kw == 2:
                        prev_colmset = nc.vector.memset(
                            ot[:, OM + 63:OM + L:64], 0.0)

                    deng = nc.sync if kk % 2 == 0 else nc.scalar
                    if os.environ.get("K_STORE_ENG") == "sync":
                        deng = nc.sync
                    deng.dma_start(out_ext[rows, kk * L:(kk + 1) * L],
                                   ot[:, OM:OM + L])
    nc.compile()
    return nc


_GRAPH_CACHE = {}


def _get_graph():
    if "nc" not in _GRAPH_CACHE:
        _GRAPH_CACHE["nc"] = build_graph()
    return _GRAPH_CACHE["nc"]


def kernel(key_map: np.ndarray, query_map: np.ndarray,
           _trace: bool = False, _tmpdir: str | None = None):
    key_map = np.ascontiguousarray(key_map, dtype=np.float32)
    query_map = np.ascontiguousarray(query_map, dtype=np.float32)
    assert key_map.shape == (B, C, H, W), key_map.shape

    nc = _get_graph()
    in_maps = [
        {"key_map": key_map[b].reshape(C, L),
         "query_map": query_map[b].reshape(C, L)}
        for b in range(B)
    ]
    res = run_bass_kernel_spmd(
        nc, in_maps, core_ids=list(range(B)),
        trace=_trace, tmpdir=_tmpdir,
    )
    out = np.stack([res.results[b]["out"] for b in range(B)])
    _GRAPH_CACHE["last_exec_time_ns"] = res.exec_time_ns
    _GRAPH_CACHE["last_results"] = res
    return out.reshape(B, C, L, K2)


# revision 21
# speedup vs baseline: 1.2521x; 1.0343x over previous
"""Trainium2 Bass kernel for nn_AppearanceComposability (sparse_attention).

Reference semantics, per (b, c) with 64x64 images, 3x3 unfold (pad 1):
  key_uf  = unfold(key)[b]  : [C*9, 4096]   (channel order (C, kh, kw))
  out     = key_uf.view(C, 4096, 9) * query_uf.view(C, 4096, 9)[..., 4:5]
The raw .view interleave means, with K_flat = per-channel flattened patch
block (kk*4096 + l) and similarly Q_flat:
  out_flat[m] = K_flat[m] * qv[m // 9],   qv[i] = Q_flat[9*i + 4]

Implementation, per NeuronCore (one batch of 8), per 128-channel group
(channels on partitions):
  - load key/query 64x64 images into zero-margined SBUF buffers
  - qv built with 9 stride-9 copies from the query buffer (margins supply
    the unfold zero padding) + small strided memsets for the x-edge wraps
  - per kk chunk: one tensor_tensor multiply of the shifted key image with
    a repeat-9 "stretched" qv (step-0 access pattern), group-aligned with
    margin over/underhang; then a stride-64 memset for x-edge columns
  - 9 stores of [128, 4096] per group to the output chunk

Data parallel over batch: 8 cores, core b handles batch b. No collectives.
"""
import os
import sys

import numpy as np


def _ensure_path():
    try:
        import concourse  # noqa: F401
    except ImportError:
        for p in ("/opt/trn_rl_repo", "/root/.axon_site/_ro/trn_rl_repo"):
            if os.path.isdir(p):
                sys.path.insert(0, p)
                return


_ensure_path()

import concourse.bacc as bacc  # noqa: E402
import concourse.tile as tile  # noqa: E402
from concourse import mybir  # noqa: E402
from concourse.bass_utils import run_bass_kernel_spmd  # noqa: E402
from concourse.tile import add_dep_helper  # noqa: E402


def _install_ntff_hook_shim():
    """Provide antenv.axon_hooks when the image's antenv lacks it.

    concourse.bass_utils imports it unconditionally on the trace path; the
    boot script degrades silently when it is missing. This shim recreates
    the documented hook using the same ctypes loader the boot script uses.
    """
    try:
        import antenv.axon_hooks  # noqa: F401
        return
    except ImportError:
        pass
    try:
        import types

        import antenv
        holder = {"hook": None, "tried": False}

        def set_axon_ntff_profile_hook(h):
            holder["hook"] = h
            holder["tried"] = True

        def get_axon_ntff_profile_hook():
            if not holder["tried"]:
                holder["tried"] = True
                try:
                    from trn_agent_boot.trn_boot import _ntff_profile_via_ctypes
                    so = "/opt/axon/libaxon_pjrt.so"
                    if os.path.exists(so):
                        holder["hook"] = _ntff_profile_via_ctypes(so)
                except Exception:
                    holder["hook"] = None
            return holder["hook"]

        mod = types.ModuleType("antenv.axon_hooks")
        mod.set_axon_ntff_profile_hook = set_axon_ntff_profile_hook
        mod.get_axon_ntff_profile_hook = get_axon_ntff_profile_hook
        sys.modules["antenv.axon_hooks"] = mod
        antenv.axon_hooks = mod
    except Exception:
        pass


_install_ntff_hook_shim()

F32 = mybir.dt.float32

B = 8          # batch == number of cores
C = 256        # channels
H = W = 64
L = H * W      # 4096 pixels
K2 = 9         # 3x3 patch
M = L * K2     # 36864 per-channel output length
MARG = 80      # input image margin (>= 73 needed)
OM = 8         # output tile margin (>= 8 needed)
OFFS = [(kh - 1) * W + (kw - 1) for kh in range(3) for kw in range(3)]

# All multiplies run on DVE. GpSimd tensor_tensor shares an SBUF port pair
# with DVE's second read port; concurrent DVE-TT + GpSimd-TT measured ~2.5x
# slower on both engines (exclusive port lock), so offloading to GpSimd is
# a net loss. DVE alone (~81 us) stays below the DMA floor (~110 us).
POOL_KKS = ()


def _ceil_div(a, b):
    return -(-a // b)


def _plan_qv_ops():
    """Per kk: (i_lo, i_hi, src_start, memsets) for qv[i] = Q_flat[9i+4]."""
    ops = []
    for kk in range(K2):
        s = L * kk
        i_lo = max(0, _ceil_div(s - 4, 9))
        i_hi = min(L, _ceil_div(s + L - 4, 9))
        src_start = 9 * i_lo + 4 - s + OFFS[kk]
        memsets = []
        kw = kk % 3
        if kw != 1:
            target = 0 if kw == 0 else 63
            i0 = (57 * (target - 4 + s)) % 64  # 57 = 9^-1 mod 64
            first = i_lo + ((i0 - i_lo) % 64)
            if first < i_hi:
                cnt = (i_hi - 1 - first) // 64 + 1
                memsets.append((first, cnt, 64))
        ops.append((i_lo, i_hi, src_start, memsets))
    return ops


def _plan_tt_ops():
    """Per kk: (g_lo, g_hi, ngroups, q0); TT covers l in [g_lo, g_hi)."""
    ops = []
    for kk in range(K2):
        s = L * kk
        g_lo = -(s % 9)
        g_hi = L + ((-(s + L)) % 9)
        ops.append((g_lo, g_hi, (g_hi - g_lo) // 9, (s + g_lo) // 9))
    return ops


QV_OPS = _plan_qv_ops()
TT_OPS = _plan_tt_ops()


def build_graph():
    nc = bacc.Bacc(None, target_bir_lowering=False)
    key_ext = nc.declare_dram_parameter("key_map", [C, L], F32, isOutput=False)
    query_ext = nc.declare_dram_parameter("query_map", [C, L], F32, isOutput=False)
    out_ext = nc.declare_dram_parameter("out", [C, M], F32, isOutput=True)

    ngroups = C // 128
    with tile.TileContext(nc) as tc:
        with (
            tc.tile_pool(name="pads", bufs=1) as pads,
            tc.tile_pool(name="qvp", bufs=1) as qvp,
            tc.tile_pool(name="outs", bufs=6) as outs,
        ):
            key_pads, q_pads, qvs = [], [], []
            # Phase 0: prefetch inputs and build both groups' qv on ACT
            # before any store is queued. Ring discipline learned from
            # profiles: (a) concurrent DMAs split the 16 SDMA engines at
            # packet granularity — exactly two in flight (one per HWDGE
            # ring) gives ~420 GB/s aggregate, one alone only ~240; (b) the
            # pipeline-fill gate is q0 (qv copies) then k0 (first TT), so k0
            # is split into two chained halves — q0 then finishes ~3us
            # before the k0 tail instead of tied with it; (c) the second
            # group's load *issues* are placed after the first group's ACT
            # copies so the ACT sequencer is never blocked on a load
            # semaphore ahead of the copies.
            for g in range(ngroups):
                q_pad = pads.tile([128, MARG + L + MARG], F32,
                                  name=f"q_pad{g}", tag=f"q_pad{g}")
                nc.vector.memset(q_pad[:, 0:MARG], 0.0)
                nc.vector.memset(q_pad[:, MARG + L:MARG + L + MARG], 0.0)
                key_pad = pads.tile([128, MARG + L + MARG], F32,
                                    name=f"key_pad{g}", tag=f"key_pad{g}")
                nc.vector.memset(key_pad[:, 0:MARG], 0.0)
                nc.vector.memset(key_pad[:, MARG + L:MARG + L + MARG], 0.0)
                key_pads.append(key_pad)
                q_pads.append(q_pad)

            q0dma = nc.sync.dma_start(q_pads[0][:, MARG:MARG + L],
                                      query_ext[0:128, :])
            k0dma = nc.scalar.dma_start(key_pads[0][:, MARG:MARG + L],
                                        key_ext[0:128, :])

            def emit_qv(g):
                qv = qvp.tile([128, L], F32, name=f"qv{g}", tag=f"qv{g}")
                for kk in range(K2):
                    i_lo, i_hi, src_start, _ = QV_OPS[kk]
                    n = i_hi - i_lo
                    src0 = MARG + src_start
                    nc.scalar.copy(qv[:, i_lo:i_hi],
                                   q_pads[g][:, src0:src0 + 9 * n:9])
                qvs.append(qv)

            emit_qv(0)

            q1dma = nc.sync.dma_start(q_pads[1][:, MARG:MARG + L],
                                      query_ext[128:256, :])
            add_dep_helper(q1dma.ins, q0dma.ins, sync=True,
                           reason="serialize group loads on ring")
            k1dma = nc.scalar.dma_start(key_pads[1][:, MARG:MARG + L],
                                        key_ext[128:256, :])
            add_dep_helper(k1dma.ins, k0dma.ins, sync=True,
                           reason="serialize group loads on ring")

            emit_qv(1)

            # Phase 1: per group — the nine chunk multiplies. The qv edge
            # memset for chunk kk is emitted just before TT kk (TT kk's qv
            # read range only ever overlaps chunk kk's own memset), so DVE
            # never stalls waiting for late ACT copies.
            prev_colmset = None
            for g in range(ngroups):
                rows = slice(g * 128, (g + 1) * 128)
                key_pad, qv = key_pads[g], qvs[g]

                for kk in range(K2):
                    for (first, cnt, stride) in QV_OPS[kk][3]:
                        nc.vector.memset(
                            qv[:, first:first + (cnt - 1) * stride + 1:stride],
                            0.0)
                    g_lo, g_hi, ng, q0 = TT_OPS[kk]
                    ot = outs.tile([128, OM + L + OM], F32,
                                   name=f"ot{g}_{kk}", tag="ot")
                    dst = ot[:, OM + g_lo:OM + g_hi].rearrange(
                        "p (n k) -> p n k", k=9)
                    src_k = key_pad[:, MARG + g_lo + OFFS[kk]:
                                    MARG + g_hi + OFFS[kk]].rearrange(
                        "p (n k) -> p n k", k=9)
                    src_q = qv[:, q0:q0 + ng].unsqueeze(2).broadcast_to(
                        [128, ng, 9])
                    eng = nc.gpsimd if kk in POOL_KKS else nc.vector
                    tt = eng.tensor_mul(dst, src_k, src_q)
                    if prev_colmset is not None:
                        # Pin DVE order TT_k -> colmset_k -> TT_{k+1}: the
                        # scheduler otherwise runs the next TT first, holding
                        # the finished tile's store back a full TT (~4.4us).
                        add_dep_helper(tt.ins, prev_colmset.ins, sync=False,
                                       reason="colmset before next TT")
                        prev_colmset = None

                    kw = kk % 3
                    if kw == 0:
                        prev_colmset = nc.vector.memset(
                            ot[:, OM:OM + L:64], 0.0)
                    elif kw == 2:
                        prev_colmset = nc.vector.memset(
                            ot[:, OM + 63:OM + L:64], 0.0)

                    deng = nc.sync if kk % 2 == 0 else nc.scalar
                    if os.environ.get("K_STORE_ENG") == "sync":
                        deng = nc.sync
                    deng.dma_start(out_ext[rows, kk * L:(kk + 1) * L],
                                   ot[:, OM:OM + L])
    nc.compile()
    return nc


_GRAPH_CACHE = {}


def _get_graph():
    if "nc" not in _GRAPH_CACHE:
        _GRAPH_CACHE["nc"] = build_graph()
    return _GRAPH_CACHE["nc"]


def kernel(key_map: np.ndarray, query_map: np.ndarray,
           _trace: bool = False, _tmpdir: str | None = None):
    key_map = np.ascontiguousarray(key_map, dtype=np.float32)
    query_map = np.ascontiguousarray(query_map, dtype=np.float32)
    assert key_map.shape == (B, C, H, W), key_map.shape

    nc = _get_graph()
    in_maps = [
        {"key_map": key_map[b].reshape(C, L),
         "query_map": query_map[b].reshape(C, L)}
        for b in range(B)
    ]
    res = run_bass_kernel_spmd(
        nc, in_maps, core_ids=list(range(B)),
        trace=_trace, tmpdir=_tmpdir,
    )
    out = np.stack([res.results[b]["out"] for b in range(B)])
    _GRAPH_CACHE["last_exec_time_ns"] = res.exec_time_ns
    _GRAPH_CACHE["last_results"] = res
    return out.reshape(B, C, L, K2)
